# revision 20
# baseline (speedup 1.0000x reference)
"""Trainium2 Bass kernel for the EnhancedBCMLayer (block-circulant matrix layer).

Math: out[B, 16f+i] = sum_{g,j} iv[f,g,(i-j)%16] * x[B,16g+j] + b[16f+i]
i.e. per (f,g) 16x16 block the weight is circulant. Computed in the rfft
domain: for each of the 9 rfft bins k, Yhat_k[B,f] = sum_g Phat_k[f,g] *
Xhat_k[B,g] (complex). The cheap length-16 rfft/irfft transforms run on the
host; the expensive einsum over g runs on 8 NeuronCores (data-parallel over
the batch), packed as 32 matmuls of [128,128] @ [128,512]:

  - complex bins pair (Re,Im) components; contraction K = (2 comps x 64 g),
    output M = (2 comps x 64 f), with the 2x2 complex-multiply block structure
    baked into the host-built stationary weights.
  - the two real bins (0 and 8) share one pair slot with a block-diagonal
    weight.

Data movement runs at the serialized-DMA-transfer floor: per pair, the rhs
tile and its 4 weight tiles are packed into one contiguous per-partition
stream (one DMA per chunk of pairs, 3KB contiguous runs), fp16 end-to-end
(fp16 keeps 10 mantissa bits; PSUM accumulates fp32, so the result is within
~4e-4 of the fp32 reference while moving half the bytes).
"""

import numpy as np
import ml_dtypes

import concourse.mybir as mybir
import concourse.tile as tile
from concourse import bacc
from concourse.bass_utils import run_bass_kernel_spmd

N_CORES = 8
BATCH = 4096
IN_FEATURES = 2048
OUT_FEATURES = 2048
BS = 16          # circulant block size
NB = 128         # feature blocks (f and g)
BINS = 9         # rfft bins of length-16 signal
NPAIR = 8        # component pairs: (re0,re8), (re1,im1), ..., (re7,im7)
BC = BATCH // N_CORES  # 512 batch rows per core
CHUNKS = [(0, 1), (1, 1), (2, 2), (4, 2), (6, 2)]  # (first pair, npairs) per DMA chunk

# dtype config: matmul operand dtype and device-output dtype
XDT = mybir.dt.float16
ODT = mybir.dt.float16

_DT_NP = {
    mybir.dt.float32r: np.float32,
    mybir.dt.float32: np.float32,
    mybir.dt.bfloat16: ml_dtypes.bfloat16,
    mybir.dt.float16: np.float16,
}

_CACHED = {}
NWARM = 6        # dummy PE-warmup matmuls issued during the initial DMA wait


def _emit_body(nc, tc, pools, xwin, yout, xdt, odt, warm=0):
    f32 = mybir.dt.float32
    xp, op, ps = pools
    # Interleave x-chunk and w-chunk DMAs so the first pair's matmuls start as
    # early as possible (transfers serialize on the DMA engines); chunks are
    # small at the start to shorten the pipeline ramp.
    XW = 2 * BC + 4 * 128  # packed per-pair row: x (2*BC) then w (4*128)
    xwchunks = []
    for c, (p0, npair) in enumerate(CHUNKS):
        xwc = xp.tile([128, npair, XW], xdt, tag=f"xw{c}")
        nc.sync.dma_start(xwc[:], xwin[p0:p0 + npair].rearrange("p k e -> k p e"))
        xwchunks.append(xwc)
    if warm:
        # dummy matmuls on a zeroed scratch tile keep the PE HAM-warm while
        # the first input DMAs are in flight, so real matmuls run at 2.4GHz
        z = xp.tile([128, 512], xdt, tag="warmz")
        nc.gpsimd.memset(z[:], 0.0)
        wps = tc.warm_pool.tile([128, 512], f32, tag="warmp")
        for _ in range(warm):
            nc.tensor.matmul(wps[:], z[:, :128], z[:], start=True, stop=True)
    for c, (p0, npair) in enumerate(CHUNKS):
        # copy PSUM->SBUF (alternating DVE/ACT), then DMA out per pair.
        oc = op.tile([128, npair, 2, BC], odt, tag=f"o{c}")
        for pp in range(npair):
            acc = ps.tile([128, 2, BC], f32, tag="acc")
            for fh in range(2):
                for gh in range(2):
                    t = fh * 2 + gh
                    nc.tensor.matmul(acc[:, fh],
                                     xwchunks[c][:, pp, 2 * BC + t * 128:
                                                  2 * BC + (t + 1) * 128],
                                     xwchunks[c][:, pp, gh * BC:(gh + 1) * BC],
                                     start=(gh == 0), stop=(gh == 1))
            if (p0 + pp) % 2 == 0:
                nc.vector.tensor_copy(out=oc[:, pp], in_=acc[:])
            else:
                nc.scalar.copy(out=oc[:, pp], in_=acc[:])
            nc.sync.dma_start(yout[p0 + pp], oc[:, pp])


def _build_nc(loop_reps=0, xdt=None, odt=None):
    """Build the Bass program (one NEFF, SPMD across 8 cores).

    loop_reps > 0 wraps the body in a For_i loop running it that many times
    (benchmarking variant; output identical since iterations are idempotent).
    """
    xdt = xdt or XDT
    odt = odt or ODT
    nc = bacc.Bacc("TRN2", target_bir_lowering=False, num_devices=N_CORES)
    xwin = nc.dram_tensor("xwin", [NPAIR, 128, 2 * BC + 4 * 128], xdt,
                          kind="ExternalInput")
    yout = nc.dram_tensor("yout", [NPAIR, 128, 2, BC], odt,
                          kind="ExternalOutput")

    with tile.TileContext(nc) as tc:
        with (
            tc.tile_pool(name="xp", bufs=1) as xp,
            tc.tile_pool(name="op", bufs=2) as op,
            tc.tile_pool(name="ps", bufs=3, space="PSUM") as ps,
            tc.tile_pool(name="warmps", bufs=1, space="PSUM") as warm_pool,
        ):
            tc.warm_pool = warm_pool
            pools = (xp, op, ps)
            if loop_reps:
                with tc.For_i(0, loop_reps, 1, staggered_reset=True):
                    _emit_body(nc, tc, pools, xwin, yout, xdt, odt)
            else:
                _emit_body(nc, tc, pools, xwin, yout, xdt, odt, warm=NWARM)
    nc.compile()
    return nc


def _host_prep_weights(index_vectors, xdt=None):
    """Host: rfft the circulant generators and pack the stationary weights
    win[K=(cin*64+g'), pair, fh, gh, M=(cout*64+f')]."""
    xdt = xdt or XDT
    Phat = np.fft.rfft(index_vectors.astype(np.float64), axis=-1)  # (f,g,9)
    win = np.zeros((NPAIR, 2, 2, 128, 128), dtype=np.float64)
    for p in range(NPAIR):
        for fh in range(2):
            for gh in range(2):
                fs = slice(64 * fh, 64 * fh + 64)
                gs = slice(64 * gh, 64 * gh + 64)
                if p == 0:
                    win[p, fh, gh, 0:64, 0:64] = Phat[fs, gs, 0].real.T  # [g',f']
                    win[p, fh, gh, 64:128, 64:128] = Phat[fs, gs, 8].real.T
                else:
                    pr = Phat[fs, gs, p].real.T
                    pi = Phat[fs, gs, p].imag.T
                    win[p, fh, gh, 0:64, 0:64] = pr      # Xr -> Yr
                    win[p, fh, gh, 64:128, 0:64] = -pi   # Xi -> Yr
                    win[p, fh, gh, 0:64, 64:128] = pi    # Xr -> Yi
                    win[p, fh, gh, 64:128, 64:128] = pr  # Xi -> Yi
    # [pair, fh, gh, K, M] -> [pair, K, (fh gh), M]
    win = win.reshape(NPAIR, 4, 128, 128).transpose(0, 2, 1, 3)
    return np.ascontiguousarray(win.astype(_DT_NP[xdt]))


def _host_prep_x(x, xdt=None):
    """Host: rfft the input blocks and lay out per-core rhs
    xin[K=(comp*64+g'), pair, gh, b]."""
    xdt = xdt or XDT
    Xf = np.fft.rfft(x.reshape(BATCH, NB, BS), axis=-1)  # (B, g, 9) complex128
    xin = np.empty((N_CORES, NPAIR, 2, 2, 64, BC), dtype=np.float64)
    XfT = Xf.transpose(1, 2, 0)  # (g, bin, B)
    for p in range(NPAIR):
        if p == 0:
            c0 = XfT[:, 0].real
            c1 = XfT[:, 8].real
        else:
            c0 = XfT[:, p].real
            c1 = XfT[:, p].imag
        for gh in range(2):
            gs = slice(64 * gh, 64 * gh + 64)
            for core in range(N_CORES):
                bsl = slice(core * BC, (core + 1) * BC)
                xin[core, p, gh, 0] = c0[gs, bsl]
                xin[core, p, gh, 1] = c1[gs, bsl]
    # [core, pair, gh, K=(comp,g'), b] -> [core, pair, K, gh, b]
    xin = xin.reshape(N_CORES, NPAIR, 2, 128, BC).transpose(0, 1, 3, 2, 4)
    return np.ascontiguousarray(xin.astype(_DT_NP[xdt]))


def _host_post(youts, b):
    """Host: reassemble Yhat bins from the 8 cores' outputs, irfft, add bias."""
    Yf = np.empty((BATCH, NB, BINS), dtype=np.complex128)
    for core in range(N_CORES):
        # yout[pair, K=(cout,f'), fh, b] -> [pair, fh, K, b]
        y = np.asarray(youts[core]).astype(np.float64).transpose(0, 2, 1, 3)
        bsl = slice(core * BC, (core + 1) * BC)
        yr = np.concatenate([y[:, 0, 0:64], y[:, 1, 0:64]], axis=1)    # (NPAIR,128f,BC)
        yi = np.concatenate([y[:, 0, 64:128], y[:, 1, 64:128]], axis=1)
        yrT = yr.transpose(2, 1, 0)  # (BC, f, NPAIR)
        yiT = yi.transpose(2, 1, 0)
        Yf[bsl, :, 0] = yrT[:, :, 0]
        Yf[bsl, :, 8] = yiT[:, :, 0]
        Yf[bsl, :, 1:8] = yrT[:, :, 1:] + 1j * yiT[:, :, 1:]
    out = np.fft.irfft(Yf, n=BS, axis=-1).reshape(BATCH, OUT_FEATURES)
    return (out + b.astype(np.float64)).astype(np.float32)


def run(x, index_vectors, b, trace=False):
    key = (XDT, ODT)
    if _CACHED.get("key") != key:
        _CACHED["nc"] = _build_nc()
        _CACHED["key"] = key
    nc = _CACHED["nc"]
    win = _host_prep_weights(np.asarray(index_vectors))
    xin = _host_prep_x(np.asarray(x))
    # pack per-pair x rows (2*BC) and w rows (4*128) into one stream
    dtnp = _DT_NP[XDT]
    xwin = np.empty((N_CORES, NPAIR, 128, 2 * BC + 4 * 128), dtype=dtnp)
    xwin[:, :, :, :2 * BC] = xin.reshape(N_CORES, NPAIR, 128, 2 * BC)
    xwin[:, :, :, 2 * BC:] = win.reshape(NPAIR, 128, 4 * 128)[None]
    in_maps = [{"xwin": xwin[c]} for c in range(N_CORES)]
    res = run_bass_kernel_spmd(nc, in_maps, core_ids=list(range(N_CORES)),
                               trace=trace)
    youts = [res.results[c]["yout"] for c in range(N_CORES)]
    out = _host_post(youts, np.asarray(b))
    return out, res


def kernel(x, index_vectors, b):
    out, _ = run(x, index_vectors, b)
    return out


# revision 21
# speedup vs baseline: 1.0291x; 1.0291x over previous
"""Trainium2 Bass kernel for the EnhancedBCMLayer (block-circulant matrix layer).

Math: out[B, 16f+i] = sum_{g,j} iv[f,g,(i-j)%16] * x[B,16g+j] + b[16f+i]
i.e. per (f,g) 16x16 block the weight is circulant. Computed in the rfft
domain: for each of the 9 rfft bins k, Yhat_k[B,f] = sum_g Phat_k[f,g] *
Xhat_k[B,g] (complex). The cheap length-16 rfft/irfft transforms run on the
host; the expensive einsum over g runs on 8 NeuronCores (data-parallel over
the batch), packed as 32 matmuls of [128,128] @ [128,512]:

  - complex bins pair (Re,Im) components; contraction K = (2 comps x 64 g),
    output M = (2 comps x 64 f), with the 2x2 complex-multiply block structure
    baked into the host-built stationary weights.
  - the two real bins (0 and 8) share one pair slot with a block-diagonal
    weight.

Data movement runs at the serialized-DMA-transfer floor: per pair, the rhs
tile and its 4 weight tiles are packed into one contiguous per-partition
stream (one DMA per chunk of pairs, 3KB contiguous runs), fp16 end-to-end
(fp16 keeps 10 mantissa bits; PSUM accumulates fp32, so the result is within
~4e-4 of the fp32 reference while moving half the bytes).
"""

import numpy as np
import ml_dtypes

import concourse.mybir as mybir
import concourse.tile as tile
from concourse import bacc
from concourse.bass_utils import run_bass_kernel_spmd

N_CORES = 8
BATCH = 4096
IN_FEATURES = 2048
OUT_FEATURES = 2048
BS = 16          # circulant block size
NB = 128         # feature blocks (f and g)
BINS = 9         # rfft bins of length-16 signal
NPAIR = 8        # component pairs: (re0,re8), (re1,im1), ..., (re7,im7)
BC = BATCH // N_CORES  # 512 batch rows per core
CHUNKS = [(0, 1), (1, 1), (2, 2), (4, 2), (6, 2)]  # (first pair, npairs) per DMA chunk

# dtype config: matmul operand dtype and device-output dtype
XDT = mybir.dt.float16
ODT = mybir.dt.float16

_DT_NP = {
    mybir.dt.float32r: np.float32,
    mybir.dt.float32: np.float32,
    mybir.dt.bfloat16: ml_dtypes.bfloat16,
    mybir.dt.float16: np.float16,
}

_CACHED = {}
NWARM = 6        # dummy PE-warmup matmuls issued during the initial DMA wait


def _emit_body(nc, tc, pools, xwin, yout, xdt, odt, warm=0):
    f32 = mybir.dt.float32
    xp, op, ps = pools
    # Interleave x-chunk and w-chunk DMAs so the first pair's matmuls start as
    # early as possible (transfers serialize on the DMA engines); chunks are
    # small at the start to shorten the pipeline ramp.
    XW = 2 * BC + 4 * 128  # packed per-pair row: x (2*BC) then w (4*128)
    xwchunks = []
    for c, (p0, npair) in enumerate(CHUNKS):
        xwc = xp.tile([128, npair, XW], xdt, tag=f"xw{c}")
        nc.sync.dma_start(xwc[:], xwin[p0:p0 + npair].rearrange("p k e -> k p e"))
        xwchunks.append(xwc)
    if warm:
        # dummy matmuls on a zeroed scratch tile keep the PE HAM-warm while
        # the first input DMAs are in flight, so real matmuls run at 2.4GHz
        z = xp.tile([128, 512], xdt, tag="warmz")
        nc.gpsimd.memset(z[:], 0.0)
        wps = tc.warm_pool.tile([128, 512], f32, tag="warmp")
        for _ in range(warm):
            nc.tensor.matmul(wps[:], z[:, :128], z[:], start=True, stop=True)
    for c, (p0, npair) in enumerate(CHUNKS):
        # copy PSUM->SBUF (alternating DVE/ACT), then DMA out per pair.
        oc = op.tile([128, npair, 2, BC], odt, tag=f"o{c}")
        for pp in range(npair):
            acc = ps.tile([128, 2, BC], f32, tag="acc")
            for fh in range(2):
                for gh in range(2):
                    t = fh * 2 + gh
                    nc.tensor.matmul(acc[:, fh],
                                     xwchunks[c][:, pp, 2 * BC + t * 128:
                                                  2 * BC + (t + 1) * 128],
                                     xwchunks[c][:, pp, gh * BC:(gh + 1) * BC],
                                     start=(gh == 0), stop=(gh == 1))
            if (p0 + pp) % 2 == 0:
                nc.vector.tensor_copy(out=oc[:, pp], in_=acc[:])
            else:
                nc.scalar.copy(out=oc[:, pp], in_=acc[:])
            nc.sync.dma_start(yout[p0 + pp], oc[:, pp])


def _build_nc(loop_reps=0, xdt=None, odt=None):
    """Build the Bass program (one NEFF, SPMD across 8 cores).

    loop_reps > 0 wraps the body in a For_i loop running it that many times
    (benchmarking variant; output identical since iterations are idempotent).
    """
    xdt = xdt or XDT
    odt = odt or ODT
    nc = bacc.Bacc("TRN2", target_bir_lowering=False, num_devices=N_CORES)
    xwin = nc.dram_tensor("xwin", [NPAIR, 128, 2 * BC + 4 * 128], xdt,
                          kind="ExternalInput")
    yout = nc.dram_tensor("yout", [NPAIR, 128, 2, BC], odt,
                          kind="ExternalOutput")

    with tile.TileContext(nc) as tc:
        import contextlib
        with (
            tc.tile_pool(name="xp", bufs=1) as xp,
            tc.tile_pool(name="op", bufs=2) as op,
            tc.tile_pool(name="ps", bufs=4 if loop_reps else 3,
                         space="PSUM") as ps,
            (contextlib.nullcontext() if loop_reps else
             tc.tile_pool(name="warmps", bufs=1, space="PSUM")) as warm_pool,
        ):
            tc.warm_pool = warm_pool
            pools = (xp, op, ps)
            if loop_reps:
                with tc.For_i(0, loop_reps, 1, staggered_reset=True):
                    _emit_body(nc, tc, pools, xwin, yout, xdt, odt)
            else:
                _emit_body(nc, tc, pools, xwin, yout, xdt, odt, warm=NWARM)
    nc.compile()
    return nc


def _host_prep_weights(index_vectors, xdt=None):
    """Host: rfft the circulant generators and pack the stationary weights
    win[K=(cin*64+g'), pair, fh, gh, M=(cout*64+f')]."""
    xdt = xdt or XDT
    Phat = np.fft.rfft(index_vectors.astype(np.float64), axis=-1)  # (f,g,9)
    win = np.zeros((NPAIR, 2, 2, 128, 128), dtype=np.float64)
    for p in range(NPAIR):
        for fh in range(2):
            for gh in range(2):
                fs = slice(64 * fh, 64 * fh + 64)
                gs = slice(64 * gh, 64 * gh + 64)
                if p == 0:
                    win[p, fh, gh, 0:64, 0:64] = Phat[fs, gs, 0].real.T  # [g',f']
                    win[p, fh, gh, 64:128, 64:128] = Phat[fs, gs, 8].real.T
                else:
                    pr = Phat[fs, gs, p].real.T
                    pi = Phat[fs, gs, p].imag.T
                    win[p, fh, gh, 0:64, 0:64] = pr      # Xr -> Yr
                    win[p, fh, gh, 64:128, 0:64] = -pi   # Xi -> Yr
                    win[p, fh, gh, 0:64, 64:128] = pi    # Xr -> Yi
                    win[p, fh, gh, 64:128, 64:128] = pr  # Xi -> Yi
    # [pair, fh, gh, K, M] -> [pair, K, (fh gh), M]
    win = win.reshape(NPAIR, 4, 128, 128).transpose(0, 2, 1, 3)
    return np.ascontiguousarray(win.astype(_DT_NP[xdt]))


def _host_prep_x(x, xdt=None):
    """Host: rfft the input blocks and lay out per-core rhs
    xin[K=(comp*64+g'), pair, gh, b]."""
    xdt = xdt or XDT
    Xf = np.fft.rfft(x.reshape(BATCH, NB, BS), axis=-1)  # (B, g, 9) complex128
    xin = np.empty((N_CORES, NPAIR, 2, 2, 64, BC), dtype=np.float64)
    XfT = Xf.transpose(1, 2, 0)  # (g, bin, B)
    for p in range(NPAIR):
        if p == 0:
            c0 = XfT[:, 0].real
            c1 = XfT[:, 8].real
        else:
            c0 = XfT[:, p].real
            c1 = XfT[:, p].imag
        for gh in range(2):
            gs = slice(64 * gh, 64 * gh + 64)
            for core in range(N_CORES):
                bsl = slice(core * BC, (core + 1) * BC)
                xin[core, p, gh, 0] = c0[gs, bsl]
                xin[core, p, gh, 1] = c1[gs, bsl]
    # [core, pair, gh, K=(comp,g'), b] -> [core, pair, K, gh, b]
    xin = xin.reshape(N_CORES, NPAIR, 2, 128, BC).transpose(0, 1, 3, 2, 4)
    return np.ascontiguousarray(xin.astype(_DT_NP[xdt]))


def _host_post(youts, b):
    """Host: reassemble Yhat bins from the 8 cores' outputs, irfft, add bias."""
    Yf = np.empty((BATCH, NB, BINS), dtype=np.complex128)
    for core in range(N_CORES):
        # yout[pair, K=(cout,f'), fh, b] -> [pair, fh, K, b]
        y = np.asarray(youts[core]).astype(np.float64).transpose(0, 2, 1, 3)
        bsl = slice(core * BC, (core + 1) * BC)
        yr = np.concatenate([y[:, 0, 0:64], y[:, 1, 0:64]], axis=1)    # (NPAIR,128f,BC)
        yi = np.concatenate([y[:, 0, 64:128], y[:, 1, 64:128]], axis=1)
        yrT = yr.transpose(2, 1, 0)  # (BC, f, NPAIR)
        yiT = yi.transpose(2, 1, 0)
        Yf[bsl, :, 0] = yrT[:, :, 0]
        Yf[bsl, :, 8] = yiT[:, :, 0]
        Yf[bsl, :, 1:8] = yrT[:, :, 1:] + 1j * yiT[:, :, 1:]
    out = np.fft.irfft(Yf, n=BS, axis=-1).reshape(BATCH, OUT_FEATURES)
    return (out + b.astype(np.float64)).astype(np.float32)


def run(x, index_vectors, b, trace=False):
    key = (XDT, ODT)
    if _CACHED.get("key") != key:
        _CACHED["nc"] = _build_nc()
        _CACHED["key"] = key
    nc = _CACHED["nc"]
    win = _host_prep_weights(np.asarray(index_vectors))
    xin = _host_prep_x(np.asarray(x))
    # pack per-pair x rows (2*BC) and w rows (4*128) into one stream
    dtnp = _DT_NP[XDT]
    xwin = np.empty((N_CORES, NPAIR, 128, 2 * BC + 4 * 128), dtype=dtnp)
    xwin[:, :, :, :2 * BC] = xin.reshape(N_CORES, NPAIR, 128, 2 * BC)
    xwin[:, :, :, 2 * BC:] = win.reshape(NPAIR, 128, 4 * 128)[None]
    in_maps = [{"xwin": xwin[c]} for c in range(N_CORES)]
    res = run_bass_kernel_spmd(nc, in_maps, core_ids=list(range(N_CORES)),
                               trace=trace)
    youts = [res.results[c]["yout"] for c in range(N_CORES)]
    out = _host_post(youts, np.asarray(b))
    return out, res


def kernel(x, index_vectors, b):
    out, _ = run(x, index_vectors, b)
    return out
